# revision 74
# baseline (speedup 1.0000x reference)
"""Banded multi-head attention (B=2, L=1024, D=1024, H=16, band W=64) on 8
Trainium2 NeuronCores.

Sharding: core = (batch b, head-group g): 2 batches x 4 head groups of 4
heads.  Each core: q/k/v projections for its group, banded attention for its
4 heads, partial output projection through its Wo slice; host sums the 4
partials per batch.

Kernel design:
- All matmul operands are bf16 (same 1 cycle/row PE rate as f32r at N>=256,
  but also 1 c/r at small N, and half the DMA/SBUF traffic).  PSUM stays f32.
- Inputs packed per K-chunk: one dram blob row = [x | wq | wk | wv], so the
  whole input load is 8 wide DMAs + 2 for Wo + 3 small ones (each dma_start
  costs ~625ns on the shared HWDGE, so DMA count matters).
- Attention tiled to 128-query tiles with 2 key chunks of 128 (key axis
  padded by 64: 1 zero col + 63 cache cols).
- Softmax denominator: ones-column in V gives the row-sum in the AV PSUM;
  1/denom via one DVE reciprocal, broadcast across the 64 dv partitions with
  an f32r K=1 matmul (1 c/r at N=512), one fused multiply at evacuation.
- Aux work spread across Activation / DVE / Pool(GpSimd).
- All per-rep SBUF tiles are double-buffered (bufs=2 pools) so rep N+1's
  input DMAs and projections overlap rep N's attention/output tail.
"""
import numpy as np
import ml_dtypes

import concourse.bacc as bacc
import concourse.mybir as mybir
import concourse.tile as tile
from concourse import bass_utils

B, L, D, H, W = 2, 1024, 1024, 16, 64
DH = D // H           # 64
G = 4                 # head groups
HPG = H // G          # 4 heads per group
DG = D // G           # 256 dims per group
NCORES = 8
NT = L // 128          # 8 oproj tiles of 128 tokens / 8 supers of 128 queries
KTS = 64 + L           # kT per-head stride: 64 cache (1 zero + 63) + tokens
VROW = HPG * (DH + 1)  # 260 cols per v slot (4 heads x (64 dv + ones col))
# v slots: B family j=0..7 = tokens [128j-64, 128j+64); A family (stored at
# slot index 8+j) = tokens [128j, 128j+128).  Query tile i of 64 (t0=64i)
# reads B slot i//2 when i even, A slot i//2 when i odd.
NSLOT = 16
XW = D + 3 * DG        # 1792 packed input cols

F32 = mybir.dt.float32
F32R = mybir.dt.float32r
BF16 = mybir.dt.bfloat16
NEG = -1.0e30
EXPF = mybir.ActivationFunctionType.Exp
COPYF = mybir.ActivationFunctionType.Copy
MULT = mybir.AluOpType.mult
ADD = mybir.AluOpType.add
NPBF = ml_dtypes.bfloat16


def _pin_exp_table(arch: str):
    """Keep Copy/Exp resolvable only via one act-func set so exactly one
    table load is emitted."""
    import concourse.hw_specs as hw_specs
    tables = hw_specs.get_activation_tables(arch)   # cached, mutable
    drop = {EXPF, COPYF, mybir.ActivationFunctionType.Identity}
    assert "exp_and_others" in tables
    for name, funcs in tables.items():
        if name != "exp_and_others":
            funcs -= drop


def build(repeat: int = 1, variant: str = "full", loop_n: int = 0):
    nc = bacc.Bacc("TRN2", target_bir_lowering=False, debug=False)
    _pin_exp_table(nc.m.arch)

    xin = nc.dram_tensor("xin", [D, XW], BF16, kind="ExternalInput")
    woT = nc.dram_tensor("woT", [DG, D], BF16, kind="ExternalInput")
    kc = nc.dram_tensor("kc", [DH, HPG * 64], BF16, kind="ExternalInput")
    vc = nc.dram_tensor("vc", [64, VROW], BF16, kind="ExternalInput")
    maskd = nc.dram_tensor("mask", [128, 512], BF16, kind="ExternalInput")
    seld = nc.dram_tensor("seld", [2, 128], F32R, kind="ExternalInput")
    onesb = nc.dram_tensor("onesb", [128, 64], BF16, kind="ExternalInput")
    y = nc.dram_tensor("y", [L, D], BF16, kind="ExternalOutput")

    with tile.TileContext(nc) as tc:
        with tc.tile_pool(name="res", bufs=1) as res, \
             tc.tile_pool(name="big", bufs=2) as big, \
             tc.tile_pool(name="epool", bufs=6) as epool, \
             tc.tile_pool(name="rcpool", bufs=3) as rcpool, \
             tc.tile_pool(name="ypool", bufs=3) as ypool, \
             tc.tile_pool(name="ps", bufs=3, space="PSUM") as psp, \
             tc.tile_pool(name="psa", bufs=5, space="PSUM") as psa:

            # Constants: loaded once, read-only afterwards.
            sel_sb = res.tile([2, 128], F32R, tag="sel", name="sel_sb")
            ones64 = res.tile([128, 64], BF16, tag="ones64", name="ones64")
            nc.sync.dma_start(sel_sb[:], seld.ap())
            nc.sync.dma_start(ones64[:], onesb.ap())

            import contextlib

            def rep_ctx():
                if loop_n:
                    return tc.For_i(0, loop_n, 1,
                                    hint_engines=(mybir.EngineType.PE,
                                                  mybir.EngineType.Activation,
                                                  mybir.EngineType.DVE,
                                                  mybir.EngineType.Pool,
                                                  mybir.EngineType.SP))
                return contextlib.nullcontext()

            with rep_ctx():
              for rep in range(repeat):
                  # ---- per-rep (double-buffered) SBUF tiles ---------------
                  xk = [big.tile([128, XW], BF16, tag=f"xk{k}", name=f"xk{k}")
                        for k in range(8)]
                  wo_sb = [big.tile([128, D], BF16, tag=f"wo{m}", name=f"wo{m}")
                           for m in range(2)]
                  qt = big.tile([64, 4 * L], BF16, tag="qt", name="qt")
                  kt = big.tile([64, 4 * KTS], BF16, tag="kt", name="kt")
                  v_sb = big.tile([128, NSLOT * VROW], BF16, tag="v", name="v_sb")
                  mask_sb = big.tile([128, 512], BF16, tag="mask", name="mask_sb")
                  oT = [big.tile([128, L], BF16, tag=f"oT{m}", name=f"oT{m}")
                        for m in range(2)]

                  kt4 = kt[:].rearrange("p (h c) -> p h c", c=KTS)
                  v4 = v_sb[:].rearrange("p (s h c) -> p s h c",
                                         h=HPG, c=DH + 1)

                  def emit_qk(p, m, n):
                      """p=0: q, p=1: k.  Tokens [512n,+512), heads 2m,2m+1."""
                      pt = psp.tile([128, 512], F32, tag="ps", name="pj")
                      for k in range(8):
                          nc.tensor.matmul(
                              pt[:],
                              xk[k][:, D + p * DG + m * 128:
                                    D + p * DG + m * 128 + 128],
                              xk[k][:, n * 512:(n + 1) * 512],
                              start=(k == 0), stop=(k == 7),
                          )
                      for hh in range(2):
                          h = 2 * m + hh
                          if p == 0:
                              dst = qt[:, h * L + n * 512:
                                       h * L + n * 512 + 512]
                          else:
                              dst = kt[:, h * KTS + 64 + n * 512:
                                       h * KTS + 64 + n * 512 + 512]
                          src = pt[hh * 64:(hh + 1) * 64, :]
                          if hh == 0:
                              nc.scalar.copy(dst, src)
                          else:
                              nc.vector.tensor_copy(dst, src)

                  def emit_v(t):
                      """Token block [128t,+128) -> v slot t rows 64:128 and
                      slot t+1 rows 0:64."""
                      pv = psp.tile([128, 256], F32, tag="ps", name="pjv")
                      for k in range(8):
                          nc.tensor.matmul(
                              pv[:],
                              xk[k][:, t * 128:(t + 1) * 128],
                              xk[k][:, D + 2 * DG: D + 3 * DG],
                              start=(k == 0), stop=(k == 7),
                          )
                      src = pv[:].rearrange("p (h c) -> p h c", c=DH)
                      eng = nc.scalar.copy if t % 2 else nc.vector.tensor_copy
                      eng(v4[:, 8 + t, :, 0:DH], src)

                  def emit_vb(t):
                      # B family built from A in SBUF (Pool can't touch PSUM)
                      nc.gpsimd.tensor_copy(v4[64:128, t, :, 0:DH],
                                            v4[0:64, 8 + t, :, 0:DH])
                      if t < 7:
                          nc.gpsimd.tensor_copy(v4[0:64, t + 1, :, 0:DH],
                                                v4[64:128, 8 + t, :, 0:DH])

                  sup = {}
                  rcs = {}

                  def emit_scores(s):
                      """Super s = query tiles 2s, 2s+1 (t0 = 128s)."""
                      st = psa.tile([128, 512], F32, tag="psa", name="st")
                      e = epool.tile([128, 512], BF16, tag="e", name="e")
                      for tb in range(2):  # query tile i = 2s+tb, 64 queries
                          t0 = 128 * s + 64 * tb
                          for h in range(4):
                              nc.tensor.matmul(
                                  st[:, tb * 256 + h * 64:
                                     tb * 256 + h * 64 + 64],
                                  kt[:, h * KTS + t0: h * KTS + t0 + 128],
                                  qt[:, h * L + t0: h * L + t0 + 64],
                                  start=True, stop=True,
                              )
                      nc.scalar.activation(e[:], st[:], EXPF,
                                           scale=float(DH) ** -0.5)
                      nc.gpsimd.tensor_mul(e[:], e[:], mask_sb[:])
                      sup[s] = [st, e, None, None]

                  def emit_av(s):
                      st, e, _, _ = sup[s]
                      op = psa.tile([65, 512], F32, tag="psa", name="op")
                      for tb in range(2):
                          i = 2 * s + tb
                          slot = i // 2 if i % 2 == 0 else 8 + i // 2
                          for h in range(4):
                              nc.tensor.matmul(
                                  op[0:65, tb * 256 + h * 64:
                                     tb * 256 + h * 64 + 64],
                                  v_sb[:, slot * VROW + h * (DH + 1):
                                       slot * VROW + h * (DH + 1) + DH + 1],
                                  e[:, tb * 256 + h * 64:
                                    tb * 256 + h * 64 + 64],
                                  start=True, stop=True,
                              )
                      rc = rcpool.tile([1, 512], F32R, tag="rc", name="rc")
                      with nc.vector.bass.allow_low_precision(
                              "f32r softmax denom"):
                          nc.vector.reciprocal(rc[:], op[64:65, :])
                      sup[s] = [st, e, op, rc]

                  def emit_norm(s):
                      t0 = 128 * s
                      _, _, op, rc = sup.pop(s)
                      bcp = psa.tile([64, 512], F32, tag="psa", name="bcp")
                      nc.tensor.matmul(bcp[:], sel_sb[0:1, 0:64], rc[:],
                                       start=True, stop=True)
                      bc = rcpool.tile([64, 512], F32, tag="bc", name="bc")
                      nc.scalar.copy(bc[:], bcp[:])
                      op2 = op[0:64, :].rearrange("p (t h q) -> p t h q",
                                                  t=2, q=DH)
                      bc2 = bc[:].rearrange("p (t h q) -> p t h q",
                                            t=2, q=DH)
                      for m in range(2):
                          for hh in range(2):
                              h = 2 * m + hh
                              dst = oT[m][hh * 64:(hh + 1) * 64, t0:t0 + 128]
                              nc.vector.tensor_mul(
                                  dst.rearrange("p (t q) -> p t q", t=2),
                                  op2[:, :, h, :], bc2[:, :, h, :])

                  def emit_oproj(t):
                      ysb = ypool.tile([128, D], BF16, tag="y", name="ysb")
                      for n2 in range(2):
                          yp = psp.tile([128, 512], F32, tag="ps", name="yp")
                          for m in range(2):
                              nc.tensor.matmul(
                                  yp[:],
                                  oT[m][:, t * 128:(t + 1) * 128],
                                  wo_sb[m][:, n2 * 512:(n2 + 1) * 512],
                                  start=(m == 0), stop=(m == 1),
                              )
                          sl = ysb[:, n2 * 512:(n2 + 1) * 512]
                          if n2 == 0:
                              nc.scalar.copy(sl, yp[:])
                          else:
                              nc.vector.tensor_copy(sl, yp[:])
                      nc.sync.dma_start(y.ap()[t * 128:(t + 1) * 128, :],
                                        ysb[:])

                  # ---- v ones-columns rebuilt each rep (Pool, SBUF) -------
                  vcols = v_sb[:].rearrange("p (sh c) -> p sh c", c=DH + 1)
                  nc.gpsimd.tensor_copy(vcols[:, :, DH:DH + 1],
                                        ones64[:].unsqueeze(2))
                  # ---- input DMAs ----------------------------------------
                  for k in range(8):
                      nc.sync.dma_start(xk[k][:],
                                        xin.ap()[k * 128:(k + 1) * 128, :])
                  for m in range(2):
                      nc.sync.dma_start(wo_sb[m][:],
                                        woT.ap()[m * 128:(m + 1) * 128, :])
                  nc.sync.dma_start(
                      kt4[:, :, 0:64],
                      kc.ap().rearrange("p (h c) -> p h c", c=64))
                  nc.sync.dma_start(v_sb[0:64, 0:VROW], vc.ap())
                  nc.sync.dma_start(mask_sb[:], maskd.ap())
                  # ---- compute -------------------------------------------
                  emit_qk(0, 0, 0)
                  emit_qk(1, 0, 0)
                  emit_qk(0, 1, 0)
                  emit_qk(1, 1, 0)
                  for t in range(5):
                      emit_v(t)
                      emit_vb(t)
                  emit_scores(0)
                  emit_av(0)
                  emit_scores(1)
                  emit_av(1)
                  emit_scores(2)
                  emit_norm(0)
                  emit_av(2)
                  emit_qk(0, 0, 1)
                  emit_qk(1, 0, 1)
                  emit_scores(3)
                  emit_oproj(0)
                  emit_norm(1)
                  emit_av(3)
                  emit_qk(0, 1, 1)
                  emit_qk(1, 1, 1)
                  emit_v(5)
                  emit_v(6)
                  emit_v(7)
                  emit_scores(4)
                  emit_vb(5)
                  emit_oproj(1)
                  emit_norm(2)
                  emit_av(4)
                  emit_scores(5)
                  emit_vb(6)
                  emit_oproj(2)
                  emit_norm(3)
                  emit_av(5)
                  emit_scores(6)
                  emit_vb(7)
                  emit_oproj(3)
                  emit_norm(4)
                  emit_av(6)
                  emit_scores(7)
                  emit_oproj(4)
                  emit_norm(5)
                  emit_av(7)
                  emit_norm(6)
                  emit_oproj(5)
                  emit_norm(7)
                  emit_oproj(6)
                  emit_oproj(7)

    nc.compile()
    return nc


def make_mask() -> np.ndarray:
    """[128, 256] multiplicative band mask (1 in band, 0 outside), bf16,
    applied to exp(scores) in SBUF.  Key row r = key position t0-64+r for a
    64-query tile at t0; query col j in [0,64): valid iff j+1 <= r <= j+64.
    Same for all 4 head blocks."""
    r = np.arange(128)[:, None]
    j = np.arange(64)[None, :]
    m0 = np.where((r >= j + 1) & (r <= j + 64), 1.0, 0.0).astype(np.float32)
    return np.tile(m0, (1, 8)).astype(NPBF)


def prep_inputs(x, Wq, Wk, Wv, Wo, last_k_init, last_v_init):
    """Shard + pre-transpose + bf16-cast full inputs into 8 per-core maps."""
    mask = make_mask()
    sel = np.zeros((2, 128), dtype=np.float32)
    sel[0, 0:64] = 1.0
    sel[1, 64:128] = 1.0
    in_maps = []
    for core in range(NCORES):
        b, g = divmod(core, G)
        sl = slice(g * DG, (g + 1) * DG)
        lk = last_k_init[:, g * HPG:(g + 1) * HPG, :]   # [63, 4, 64]
        lv = last_v_init[:, g * HPG:(g + 1) * HPG, :]
        xin = np.concatenate(
            [x[b].T, Wq[sl, :].T, Wk[sl, :].T, Wv[sl, :].T],
            axis=1).astype(NPBF)                        # [1024, 1792]
        # kT cache: col j (j=1..63) = key j-64 = cache idx j-1; col 0 zero
        kcg = np.zeros((DH, HPG, 64), dtype=np.float32)
        kcg[:, :, 1:64] = lk.transpose(2, 1, 0)         # [64, 4, 63]
        # v slot 0 rows 0:64 (tokens -64..-1): row 0 zero, row r = cache r-1
        vcg = np.zeros((64, HPG, DH + 1), dtype=np.float32)
        vcg[1:64, :, 0:DH] = lv
        vcg[:, :, DH] = 1.0
        in_maps.append({
            "xin": xin,
            "woT": np.ascontiguousarray(Wo[:, sl].T).astype(NPBF),
            "kc": kcg.reshape(DH, HPG * 64).astype(NPBF),
            "vc": vcg.reshape(64, VROW).astype(NPBF),
            "mask": mask,
            "seld": sel,
            "onesb": np.ones((128, 64), dtype=NPBF),
        })
    return in_maps


_built = None


def kernel(x, Wq, Wk, Wv, Wo, last_k_init, last_v_init) -> np.ndarray:
    global _built
    x = np.asarray(x, dtype=np.float32)
    args = [np.asarray(a, dtype=np.float32)
            for a in (Wq, Wk, Wv, Wo, last_k_init, last_v_init)]
    in_maps = prep_inputs(x, *args)
    if _built is None:
        _built = build()
    r = bass_utils.run_bass_kernel_spmd(
        _built, in_maps, core_ids=list(range(NCORES)))
    out = np.zeros((B, L, D), dtype=np.float32)
    for core in range(NCORES):
        b = core // G
        out[b] += np.asarray(r.results[core]["y"]).astype(np.float32)
    return out
